# revision 22
# baseline (speedup 1.0000x reference)
"""Trainium2 Bass kernel for nn_BasicTransformerBlock_18657337934637.

Sparse-attention transformer block:
  q/k/v = hidden @ W* + b*        (2304 -> 2304, 24 heads x 96)
  RoPE3D on q, k
  sparse-1d grouping (SPARSE_N=4): token t -> group t%4, 1024 tokens/group
  softmax attention within each (group, head)
  out = attn @ wo + bo

Distribution over 8 NeuronCores:
  Launch 1 (head-parallel): core c computes heads 3c..3c+2 end-to-end through
    attention.  Host pre-transposes hidden to hT [2304, 4096] in grouped token
    order (groups = contiguous 1024-token spans).  All matmul streams are
    bf16 (fp32 PSUM accumulation); rel tolerance is 2e-2 and bf16 keeps the
    full pipeline well under 1e-2.
    QKV: per 128-token sub-tile the three projections are packed into TWO
    moving-weight matmuls per contraction chunk (480+384 columns) instead of
    three 288-col ones.  Packed column layout per chunk:
      seg1[480] = [q0|k0|q1|k1|v0]   seg2[384] = [q2|k2|v1|v2]
    so each head's (q|k) pair is contiguous for rope, which reads PSUM
    directly (no staging copy).  Per (group, head): scores computed
    transposed [k, q]; exp on the scalar engine; an all-ones column in v
    yields the softmax denominator in the same PV matmul.  PSUM->SBUF
    copies run on the (otherwise idle) Pool engine.  Output: un-normalized
    attn^T + denominator row as contiguous bf16 [97, 512] blocks; the host
    divides.
  Host: normalize, gather heads -> attnT [2304, 4096], undo permutation.
  Launch 2 (token x outdim parallel): core (i, j) computes
    out[i*1024:(i+1)*1024, j*1152:(j+1)*1152]^T = wo_j^T @ attnT_i in bf16,
    as 9 rolling psum unit-pairs (no group barriers), inputs split across
    both HWDGE queues.
"""
import os
import numpy as np

HEADS = 24
HD = 96
SPN = 4
S = 4096
DIM = 2304
KC = DIM // 128            # 18 contraction chunks
HPC = 3                    # heads per core
CW = HPC * HD              # 288 projection columns per core
PK = 864                   # packed qkv columns per chunk (3*288)
G = S // SPN               # 1024 tokens per group
TB = 256                   # hT dma block (tokens)
NB = S // TB               # 16 blocks
WG = 6                     # weight dma chunk-groups (3 kc each)
SCALE = 1.0 / float(np.sqrt(HD))

_CACHE = {}
LAST_RESULTS = []          # test harness introspection


def _build_launch1(biased):
    import concourse.mybir as mybir
    import concourse.tile as tile
    from concourse import bacc
    from concourse.masks import make_identity

    f32 = mybir.dt.float32
    bf16 = mybir.dt.bfloat16
    Exp = mybir.ActivationFunctionType.Exp
    MUL = mybir.AluOpType.mult
    ADD = mybir.AluOpType.add
    nc = bacc.Bacc("TRN2", target_bir_lowering=False, debug=False)

    # all inputs host-pre-tiled to the exact SBUF layouts -> every DMA is a
    # plain 2D copy with multi-KB contiguous rows
    hT_d = nc.dram_tensor("hT", [NB, 128, KC * TB], bf16,
                          kind="ExternalInput").ap()
    wall_d = nc.dram_tensor("wall", [128, KC * PK], bf16,
                            kind="ExternalInput").ap()
    A_d = nc.dram_tensor("A", [NB, 128, 2 * 192], bf16,
                         kind="ExternalInput").ap()
    B_d = nc.dram_tensor("B", [NB, 128, 2 * 192], bf16,
                         kind="ExternalInput").ap()
    bvi_d = nc.dram_tensor("bvi", [1, HPC * (HD + 1)], f32,
                           kind="ExternalInput").ap()
    if biased:
        bqk_d = nc.dram_tensor("bqk", [1, HPC * 192], f32,
                               kind="ExternalInput").ap()
    outN_d = nc.dram_tensor("outN", [HPC, 2 * SPN, HD + 1, 512], bf16,
                            kind="ExternalOutput").ap()

    with tile.TileContext(nc) as tc:
        with (
            tc.tile_pool(name="singles", bufs=1) as singles,
            tc.tile_pool(name="hp", bufs=2) as hp,
            tc.tile_pool(name="rp", bufs=3) as rp,
            tc.tile_pool(name="qrp", bufs=3) as qrp,
            tc.tile_pool(name="tmp", bufs=2) as tmpp,
            tc.tile_pool(name="vp", bufs=16) as vp,
            tc.tile_pool(name="qtp", bufs=2) as qtp,
            tc.tile_pool(name="ktp", bufs=2) as ktp,
            tc.tile_pool(name="ep", bufs=3) as ep,
            tc.tile_pool(name="op", bufs=6) as op,
            tc.tile_pool(name="ppq", bufs=3, space="PSUM") as ppq,
            tc.tile_pool(name="ppt", bufs=1, space="PSUM") as ppt,
            tc.tile_pool(name="pps", bufs=3, space="PSUM") as pps,
            tc.tile_pool(name="ppv", bufs=1, space="PSUM") as ppv,
        ):
            _pref = {}

            def fetch_blk(blk, ht_eng=None):
                ht = hp.tile([128, KC * TB], bf16, tag="ht", name=f"ht{blk}")
                (ht_eng or nc.sync).dma_start(ht, hT_d[blk])
                a_t = rp.tile([128, 2 * 192], bf16, tag="a", name=f"a{blk}")
                nc.scalar.dma_start(a_t, A_d[blk])
                b_t = rp.tile([128, 2 * 192], bf16, tag="b", name=f"b{blk}")
                nc.scalar.dma_start(b_t, B_d[blk])
                return ht, a_t, b_t

            # block 0 rides the scalar queue while the weight groups stream
            # on sync, so the first chain's inputs arrive in parallel
            w_grp = []

            def fetch_wgrp(gi, eng):
                t = singles.tile([128, 3 * PK], bf16, tag=f"w{gi}",
                                 name=f"w{gi}")
                eng.dma_start(t, wall_d[:, gi * 3 * PK:(gi + 1) * 3 * PK])
                w_grp.append(t.rearrange("p (k c) -> p k c", k=3))

            fetch_wgrp(0, nc.sync)
            fetch_wgrp(1, nc.scalar)
            _pref[0] = fetch_blk(0)
            fetch_wgrp(2, nc.sync)
            fetch_wgrp(3, nc.scalar)
            fetch_wgrp(4, nc.sync)
            fetch_wgrp(5, nc.scalar)

            ident = singles.tile([128, 128], bf16, tag="ident", name="ident")
            make_identity(nc, ident)
            bvi_sb = singles.tile([128, HPC * (HD + 1)], f32, tag="bvi",
                                  name="bvi_sb")
            nc.gpsimd.dma_start(out=bvi_sb,
                                in_=bvi_d.to_broadcast([128, HPC * (HD + 1)]))
            if biased:
                bqk_sb = singles.tile([128, HPC * 192], f32, tag="bqk",
                                      name="bqk_sb")
                nc.gpsimd.dma_start(out=bqk_sb,
                                    in_=bqk_d.to_broadcast([128, HPC * 192]))

            qT, kT, vt, ots = {}, {}, {}, {}
            pending = []       # attention instances awaiting emission
            prev_sub = []      # sub-tiles whose transposes are not yet emitted
            ndma = [0]

            def rope_pair(src, a_s, b_s, qrp_slice):
                """qr = src*A + rot_half(src)*B for one head's [q|k] pair."""
                sv = src.rearrange("p (d c u f) -> p d c u f", d=2, c=3, u=2)
                ov = qrp_slice.rearrange("p (d c u f) -> p d c u f",
                                         d=2, c=3, u=2)
                av = a_s
                bv = b_s.rearrange("p (d c u f) -> p d c u f", d=2, c=3, u=2)
                nc.vector.tensor_tensor(ov[:, :, :, 0:1, :],
                                        sv[:, :, :, 1:2, :],
                                        bv[:, :, :, 0:1, :], MUL)
                nc.vector.tensor_tensor(ov[:, :, :, 1:2, :],
                                        sv[:, :, :, 0:1, :],
                                        bv[:, :, :, 1:2, :], MUL)
                t2 = tmpp.tile([128, 192], f32, tag="t2", name="t2")
                nc.vector.tensor_tensor(t2, src, av, MUL)
                nc.vector.tensor_tensor(qrp_slice, t2, qrp_slice, ADD)

            def emit_transposes(sub):
                g, col, qr_t = sub
                pt = ppt.tile([HD, 6 * 128], bf16, tag="pt", name="pt")
                for h in range(HPC):
                    nc.tensor.transpose(
                        pt[:, h * 128:(h + 1) * 128],
                        qr_t[:, h * 192:h * 192 + 96], ident)
                for h in range(HPC):
                    nc.tensor.transpose(
                        pt[:, (3 + h) * 128:(4 + h) * 128],
                        qr_t[:, h * 192 + 96:(h + 1) * 192], ident)
                nc.scalar.copy(
                    qT[g].rearrange("d (h t) -> d h t", h=HPC)
                    [:, :, col:col + 128],
                    pt[:, 0:384].rearrange("d (h t) -> d h t", h=HPC))
                nc.scalar.copy(
                    kT[g].rearrange("d (h t) -> d h t", h=HPC)
                    [:, :, col:col + 128],
                    pt[:, 384:768].rearrange("d (h t) -> d h t", h=HPC))

            def attn_steps(g, h, qh):
                """scoresT -> exp -> PV for one (group, head, query-half) as
                a list of emission thunks, to be interleaved between QKV
                chain matmuls so the exp handoffs hide behind them."""
                pv = ppv.tile([HD + 1, 512], f32, tag="pv",
                              name=f"pv{g}_{h}_{qh}")
                qs = qT[g][:, h * G + qh * 512:h * G + (qh + 1) * 512]
                sts = {}

                def mk_st(kc):
                    def f():
                        st = pps.tile([128, 512], f32, tag="st",
                                      name=f"st{g}_{h}_{qh}_{kc}")
                        nc.tensor.matmul(
                            st,
                            kT[g][:, h * G + kc * 128:h * G + (kc + 1) * 128],
                            qs, start=True, stop=True)
                        sts[kc] = st
                    return f

                def mk_exp_pv(kc):
                    def f():
                        ex = ep.tile([128, 512], bf16, tag="ex",
                                     name=f"ex{g}_{h}_{qh}_{kc}")
                        nc.scalar.activation(ex, sts[kc], Exp, scale=SCALE)
                        nc.tensor.matmul(
                            pv, vt[(g, kc)][:, h * 97:(h + 1) * 97], ex,
                            start=(kc == 0), stop=(kc == 7))
                    return f

                def fin():
                    ot = op.tile([HD + 1, 512], bf16, tag="ot",
                                 name=f"ot{g}_{h}_{qh}")
                    nc.vector.tensor_copy(ot, pv)
                    eng = nc.sync if ndma[0] % 2 == 0 else nc.scalar
                    ndma[0] += 1
                    eng.dma_start(outN_d[h, g * 2 + qh], ot)

                steps = [mk_st(0), mk_st(1), mk_st(2)]
                for kc in range(3, 8):
                    steps.append(mk_exp_pv(kc - 3))
                    steps.append(mk_st(kc))
                steps.extend([mk_exp_pv(kc) for kc in range(5, 8)])
                steps.append(fin)
                return steps

            def attn_instance(g, h, qh):
                for f in attn_steps(g, h, qh):
                    f()

            for blk in range(NB):
                g = blk // 4
                if blk % 4 == 0:
                    qT[g] = qtp.tile([HD, HPC * G], bf16, tag="qT",
                                     name=f"qT{g}")
                    kT[g] = ktp.tile([HD, HPC * G], bf16, tag="kT",
                                     name=f"kT{g}")
                ht, a_t, b_t = _pref.pop(blk) if blk in _pref else fetch_blk(blk)
                if blk + 1 < NB:
                    _pref[blk + 1] = fetch_blk(blk + 1)
                htv = ht.rearrange("p (k t) -> p k t", k=KC)

                for sub in range(2):
                    tb = blk * 2 + sub
                    col = (tb % 8) * 128
                    # transposes for the previous sub-tile FIRST: the
                    # interleaved attention matmuls below may read kT slices
                    # they produce, and the PE is in-order
                    if len(prev_sub) > 1:
                        emit_transposes(prev_sub.pop(0))
                    # one pending attention instance, interleaved step-wise
                    # between the chain matmuls (each step's producer then
                    # has ~2 matmuls of slack before its consumer).  Not on
                    # the first sub of a group: the previous group's last
                    # transposes (2-sub delay) are only emitted above at the
                    # SECOND sub, and the instance reads that kT region.
                    steps = (attn_steps(*pending.pop(0))
                             if pending and tb % 8 != 0 else [])
                    si = 0
                    # packed QKV as two sequential accumulation chains, so
                    # the second chain's psum slot (reused from the previous
                    # sub-tile) has a full chain's time to be consumed
                    ps1 = ppq.tile([128, 480], f32, tag="ps", name=f"ps1_{tb}")
                    ps2 = ppq.tile([128, 384], f32, tag="ps", name=f"ps2_{tb}")
                    for kc in range(KC):
                        nc.tensor.matmul(ps1,
                                         htv[:, kc, sub * 128:(sub + 1) * 128],
                                         w_grp[kc // 3][:, kc % 3, 0:480],
                                         start=(kc == 0), stop=(kc == KC - 1))
                        if kc % 2 == 1 and si < len(steps):
                            steps[si](); si += 1
                    for kc in range(KC):
                        nc.tensor.matmul(ps2,
                                         htv[:, kc, sub * 128:(sub + 1) * 128],
                                         w_grp[kc // 3][:, kc % 3, 480:PK],
                                         start=(kc == 0), stop=(kc == KC - 1))
                        if si < len(steps):
                            steps[si](); si += 1
                    while si < len(steps):
                        steps[si](); si += 1
                    # rope on the three contiguous [q|k] psum pairs
                    qr_t = qrp.tile([128, HPC * 192], bf16, tag="qr",
                                    name=f"qr{tb}")
                    pairs = (ps1[:, 0:192], ps1[:, 192:384], ps2[:, 0:192])
                    a_s = a_t[:, sub * 192:(sub + 1) * 192]
                    b_s = b_t[:, sub * 192:(sub + 1) * 192]
                    for h, pr in enumerate(pairs):
                        src = pr
                        if biased:
                            stg = tmpp.tile([128, 192], f32, tag="stg",
                                            name=f"stg{tb}_{h}")
                            nc.vector.tensor_tensor(
                                stg, pr, bqk_sb[:, h * 192:(h + 1) * 192], ADD)
                            src = stg
                        rope_pair(src, a_s, b_s,
                                  qr_t[:, h * 192:(h + 1) * 192])
                    # V: bias add + ones column (Pool engine), cast to bf16
                    v_t = vp.tile([128, HPC * (HD + 1)], bf16, tag="v",
                                  name=f"v{tb}")
                    v3 = v_t.rearrange("p (h c) -> p h c", h=HPC)
                    bv3 = bvi_sb.rearrange("p (h c) -> p h c", h=HPC)
                    nc.vector.tensor_tensor(
                        v3[:, 0:1, 0:96], ps1[:, 384:480].rearrange(
                            "p (h c) -> p h c", h=1), bv3[:, 0:1, 0:96], ADD)
                    nc.vector.tensor_tensor(
                        v3[:, 1:3, 0:96], ps2[:, 192:384].rearrange(
                            "p (h c) -> p h c", h=2), bv3[:, 1:3, 0:96], ADD)
                    nc.gpsimd.memset(v3[:, :, 96:97], 1.0)
                    vt[(g, tb % 8)] = v_t
                    prev_sub.append((g, col, qr_t))
                if blk % 4 == 3:
                    gq = blk // 4
                    pending.extend((gq, h, qh)
                                   for h in range(HPC) for qh in range(2))
            while prev_sub:
                emit_transposes(prev_sub.pop(0))
            while pending:
                attn_instance(*pending.pop(0))
    nc.compile()
    return nc


def _build_launch2(biased):
    import concourse.mybir as mybir
    import concourse.tile as tile
    from concourse import bacc

    f32 = mybir.dt.float32
    bf16 = mybir.dt.bfloat16
    TOK = 1024           # tokens per core
    NW = 1152            # outdims per core
    MB = NW // 128       # 9 outdim blocks
    nc = bacc.Bacc("TRN2", target_bir_lowering=False, debug=False)

    at_d = nc.dram_tensor("attnT", [DIM, TOK], bf16, kind="ExternalInput").ap()
    wo_d = nc.dram_tensor("woj", [DIM, NW], bf16, kind="ExternalInput").ap()
    if biased:
        bo_d = nc.dram_tensor("boj", [1, NW], f32, kind="ExternalInput").ap()
    # transposed output [outdim, tok]; host transposes back
    out_d = nc.dram_tensor("out", [NW, TOK], bf16, kind="ExternalOutput").ap()

    with tile.TileContext(nc) as tc:
        ats, wos = [], []
        with (
            tc.tile_pool(name="singles2", bufs=1) as singles,
            tc.tile_pool(name="atp", bufs=KC) as atp,
            tc.tile_pool(name="wop", bufs=KC) as wop,
            tc.tile_pool(name="outp", bufs=3) as outp,
            tc.tile_pool(name="psp", bufs=8, space="PSUM") as psp,
        ):
            # at chunks on one HWDGE queue, wo chunks on the other
            for kc in range(KC):
                a = atp.tile([128, TOK], bf16, tag="at", name=f"at{kc}")
                nc.sync.dma_start(a, at_d[kc * 128:(kc + 1) * 128, :])
                ats.append(a)
                w = wop.tile([128, NW], bf16, tag="wo", name=f"wo{kc}")
                nc.scalar.dma_start(w, wo_d[kc * 128:(kc + 1) * 128, :])
                wos.append(w)
            if biased:
                bo_sb = singles.tile([128, MB], f32, tag="bo_sb", name="bo_sb")
                nc.sync.dma_start(bo_sb,
                                  bo_d.rearrange("a (m p) -> p (a m)", p=128))
            # kc-outer over groups of 8 units: the first group has enough
            # matmul work per chunk to stay ahead of the (bandwidth-bound)
            # 10MB input sweep, and a unit only needs chunk kc when every
            # unit does
            units = [(mb, th) for mb in range(MB) for th in range(2)]
            ot_tiles = {}
            for base in range(0, len(units), 8):
                grp = units[base:base + 8]
                pss = {u: psp.tile([128, 512], f32, tag="ps",
                                   name=f"ps{u[0]}_{u[1]}") for u in grp}
                for kc in range(KC):
                    for mb, th in grp:
                        nc.tensor.matmul(
                            pss[(mb, th)], wos[kc][:, mb * 128:(mb + 1) * 128],
                            ats[kc][:, th * 512:(th + 1) * 512],
                            start=(kc == 0), stop=(kc == KC - 1))
                for mb, th in grp:
                    if mb not in ot_tiles:
                        ot_tiles[mb] = outp.tile([128, TOK], bf16, tag="ot",
                                                 name=f"ot{mb}")
                    dst = ot_tiles[mb][:, th * 512:(th + 1) * 512]
                    if biased:
                        nc.vector.tensor_scalar_add(dst, pss[(mb, th)],
                                                    bo_sb[:, mb:mb + 1])
                    else:
                        nc.vector.tensor_copy(dst, pss[(mb, th)])
                    if th == 1:
                        eng = nc.sync if mb % 2 == 0 else nc.scalar
                        eng.dma_start(out_d[mb * 128:(mb + 1) * 128, :],
                                      ot_tiles[mb])
    nc.compile()
    return nc


def _get(name, builder, *args):
    if name not in _CACHE:
        _CACHE[name] = builder(*args)
    return _CACHE[name]


def _rope_tables(frame, height, width):
    t = np.repeat(np.arange(frame), height * width)
    y = np.tile(np.repeat(np.arange(height), width), frame)
    x = np.tile(np.arange(width), frame * height)
    D = HD // 3
    A = np.empty((S, HD), np.float32)
    B = np.empty((S, HD), np.float32)
    for i, pos in enumerate((t, y, x)):
        inv = 1.0 / (10000.0 ** (np.arange(0, D, 2, dtype=np.float32) / D))
        f = pos[:, None].astype(np.float32) * inv[None, :]
        A[:, i * D:i * D + 16] = np.cos(f)
        A[:, i * D + 16:(i + 1) * D] = np.cos(f)
        B[:, i * D:i * D + 16] = -np.sin(f)
        B[:, i * D + 16:(i + 1) * D] = np.sin(f)
    return A, B


def _tile_hT(hT, bf):
    # [2304, 4096] -> [NB, 128, KC*TB]: blk-major, partition-major, then
    # (chunk, token) contiguous per partition
    return np.ascontiguousarray(
        hT.reshape(KC, 128, NB, TB).transpose(2, 1, 0, 3).reshape(
            NB, 128, KC * TB).astype(bf))


def _tile_rope(a, bf):
    # [4096, 192] (qk-pair dup) -> [NB, 128, 2*192]
    return np.ascontiguousarray(
        a.reshape(NB, 2, 128, 192).transpose(0, 2, 1, 3).reshape(
            NB, 128, 2 * 192).astype(bf))


def _pack_wall(wq, wk, wv, sl, bf):
    # per-core packed qkv weight [2304, 864]:
    #   per chunk columns seg1=[q0|k0|q1|k1|v0] seg2=[q2|k2|v1|v2]
    q, k, v = wq[:, sl], wk[:, sl], wv[:, sl]
    h = [slice(i * 96, (i + 1) * 96) for i in range(3)]
    wall = np.concatenate(
        [q[:, h[0]], k[:, h[0]], q[:, h[1]], k[:, h[1]], v[:, h[0]],
         q[:, h[2]], k[:, h[2]], v[:, h[1]], v[:, h[2]]], axis=1)
    return np.ascontiguousarray(
        wall.reshape(KC, 128, PK).transpose(1, 0, 2).reshape(
            128, KC * PK).astype(bf))


def kernel(hidden_states, wq, bq, wk, bk, wv, bv, wo, bo, frame, height, width):
    import ml_dtypes
    from concourse import bass_utils

    bf = ml_dtypes.bfloat16
    f, hh, ww = int(frame), int(height), int(width)
    hs = np.asarray(hidden_states, dtype=np.float32)
    assert hs.shape == (1, S, DIM) and f * hh * ww == S
    wq, wk, wv, wo = (np.asarray(a, np.float32) for a in (wq, wk, wv, wo))
    bq, bk, bv, bo = (np.asarray(a, np.float32) for a in (bq, bk, bv, bo))
    biased = bool(bq.any() or bk.any())

    perm = np.concatenate([np.arange(k, S, SPN) for k in range(SPN)])
    A, B = _rope_tables(f, hh, ww)
    A = _tile_rope(np.tile(A[perm], (1, 2)), bf)
    B = _tile_rope(np.tile(B[perm], (1, 2)), bf)
    hT = _tile_hT(hs[0].T[:, perm], bf)

    nc1 = _get(f"l1_{biased}", _build_launch1, biased)
    in1 = []
    for c in range(8):
        sl = slice(c * CW, (c + 1) * CW)
        d = {
            "hT": hT,
            "wall": _pack_wall(wq, wk, wv, sl, bf),
            "bvi": np.concatenate(
                [np.concatenate([bv[sl][h * HD:(h + 1) * HD], [0.0]])
                 for h in range(HPC)]).astype(np.float32).reshape(1, -1),
            "A": A, "B": B,
        }
        if biased:
            d["bqk"] = np.concatenate(
                [np.concatenate([bq[sl][h * HD:(h + 1) * HD],
                                 bk[sl][h * HD:(h + 1) * HD]])
                 for h in range(HPC)]).astype(np.float32).reshape(1, -1)
        in1.append(d)
    td = os.environ.get("KERNEL_TRACE_DIR")
    if td:
        os.makedirs(td + "/l1", exist_ok=True)
        for fn in os.listdir(td + "/l1"):
            os.unlink(td + "/l1/" + fn)
    res1 = bass_utils.run_bass_kernel_spmd(
        nc1, in1, core_ids=list(range(8)),
        tmpdir=(td + "/l1") if td else None)
    LAST_RESULTS.append(res1)

    # outN [HPC, 8, 97, 512] bf16 -> [HPC*97, 4096] fp32, normalize, unpermute
    outN = np.concatenate(
        [np.asarray(res1.results[c]["outN"]).astype(np.float32)
         .transpose(0, 2, 1, 3).reshape(HPC, HD + 1, S) for c in range(8)], 0
    ).reshape(8 * HPC, HD + 1, S)
    attnT_g = (outN[:, :HD, :] / outN[:, HD:HD + 1, :]).reshape(DIM, S)
    attnT = np.empty_like(attnT_g)
    attnT[:, perm] = attnT_g

    biased2 = bool(bo.any())
    nc2 = _get(f"l2_{biased2}", _build_launch2, biased2)
    in2 = []
    for c in range(8):
        i, j = divmod(c, 2)
        d2 = {
            "attnT": np.ascontiguousarray(
                attnT[:, i * 1024:(i + 1) * 1024].astype(bf)),
            "woj": np.ascontiguousarray(
                wo[:, j * 1152:(j + 1) * 1152].astype(bf)),
        }
        if biased2:
            d2["boj"] = np.ascontiguousarray(
                bo[j * 1152:(j + 1) * 1152]).reshape(1, 1152)
        in2.append(d2)
    if td:
        os.makedirs(td + "/l2", exist_ok=True)
        for fn in os.listdir(td + "/l2"):
            os.unlink(td + "/l2/" + fn)
    res2 = bass_utils.run_bass_kernel_spmd(
        nc2, in2, core_ids=list(range(8)),
        tmpdir=(td + "/l2") if td else None)
    LAST_RESULTS.append(res2)

    out = np.empty((S, DIM), np.float32)
    for c in range(8):
        i, j = divmod(c, 2)
        out[i * 1024:(i + 1) * 1024, j * 1152:(j + 1) * 1152] = \
            np.asarray(res2.results[c]["out"]).T.astype(np.float32)
    return out[None]


# revision 26
# speedup vs baseline: 1.0020x; 1.0020x over previous
"""Trainium2 Bass kernel for nn_BasicTransformerBlock_18657337934637.

Sparse-attention transformer block:
  q/k/v = hidden @ W* + b*        (2304 -> 2304, 24 heads x 96)
  RoPE3D on q, k
  sparse-1d grouping (SPARSE_N=4): token t -> group t%4, 1024 tokens/group
  softmax attention within each (group, head)
  out = attn @ wo + bo

Distribution over 8 NeuronCores:
  Launch 1 (head-parallel): core c computes heads 3c..3c+2 end-to-end through
    attention.  Host pre-transposes hidden to hT [2304, 4096] in grouped token
    order (groups = contiguous 1024-token spans).  All matmul streams are
    bf16 (fp32 PSUM accumulation); rel tolerance is 2e-2 and bf16 keeps the
    full pipeline well under 1e-2.
    QKV: per 128-token sub-tile the three projections are packed into TWO
    moving-weight matmuls per contraction chunk (480+384 columns) instead of
    three 288-col ones.  Packed column layout per chunk:
      seg1[480] = [q0|k0|q1|k1|v0]   seg2[384] = [q2|k2|v1|v2]
    so each head's (q|k) pair is contiguous for rope, which reads PSUM
    directly (no staging copy).  Per (group, head): scores computed
    transposed [k, q]; exp on the scalar engine; an all-ones column in v
    yields the softmax denominator in the same PV matmul.  PSUM->SBUF
    copies run on the (otherwise idle) Pool engine.  Output: un-normalized
    attn^T + denominator row as contiguous bf16 [97, 512] blocks; the host
    divides.
  Host: normalize, gather heads -> attnT [2304, 4096], undo permutation.
  Launch 2 (token x outdim parallel): core (i, j) computes
    out[i*1024:(i+1)*1024, j*1152:(j+1)*1152]^T = wo_j^T @ attnT_i in bf16,
    as 9 rolling psum unit-pairs (no group barriers), inputs split across
    both HWDGE queues.
"""
import os
import numpy as np

HEADS = 24
HD = 96
SPN = 4
S = 4096
DIM = 2304
KC = DIM // 128            # 18 contraction chunks
HPC = 3                    # heads per core
CW = HPC * HD              # 288 projection columns per core
PK = 864                   # packed qkv columns per chunk (3*288)
G = S // SPN               # 1024 tokens per group
TB = 256                   # hT dma block (tokens)
NB = S // TB               # 16 blocks
WG = 6                     # weight dma chunk-groups (3 kc each)
SCALE = 1.0 / float(np.sqrt(HD))

_CACHE = {}
LAST_RESULTS = []          # test harness introspection


def _build_launch1(biased):
    import concourse.mybir as mybir
    import concourse.tile as tile
    from concourse import bacc
    from concourse.masks import make_identity

    f32 = mybir.dt.float32
    bf16 = mybir.dt.bfloat16
    Exp = mybir.ActivationFunctionType.Exp
    MUL = mybir.AluOpType.mult
    ADD = mybir.AluOpType.add
    nc = bacc.Bacc("TRN2", target_bir_lowering=False, debug=False)

    # all inputs host-pre-tiled to the exact SBUF layouts -> every DMA is a
    # plain 2D copy with multi-KB contiguous rows
    hT_d = nc.dram_tensor("hT", [NB, 128, KC * TB], bf16,
                          kind="ExternalInput").ap()
    wall_d = nc.dram_tensor("wall", [128, KC * PK], bf16,
                            kind="ExternalInput").ap()
    A_d = nc.dram_tensor("A", [NB, 128, 2 * 192], bf16,
                         kind="ExternalInput").ap()
    B_d = nc.dram_tensor("B", [NB, 128, 2 * 192], bf16,
                         kind="ExternalInput").ap()
    bvi_d = nc.dram_tensor("bvi", [1, HPC * (HD + 1)], f32,
                           kind="ExternalInput").ap()
    if biased:
        bqk_d = nc.dram_tensor("bqk", [1, HPC * 192], f32,
                               kind="ExternalInput").ap()
    outN_d = nc.dram_tensor("outN", [HPC, 2 * SPN, HD + 1, 512], bf16,
                            kind="ExternalOutput").ap()

    with tile.TileContext(nc) as tc:
        with (
            tc.tile_pool(name="singles", bufs=1) as singles,
            tc.tile_pool(name="hp", bufs=2) as hp,
            tc.tile_pool(name="rp", bufs=3) as rp,
            tc.tile_pool(name="qrp", bufs=3) as qrp,
            tc.tile_pool(name="tmp", bufs=2) as tmpp,
            tc.tile_pool(name="vp", bufs=16) as vp,
            tc.tile_pool(name="qtp", bufs=2) as qtp,
            tc.tile_pool(name="ktp", bufs=2) as ktp,
            tc.tile_pool(name="ep", bufs=3) as ep,
            tc.tile_pool(name="op", bufs=6) as op,
            tc.tile_pool(name="ppq", bufs=3, space="PSUM") as ppq,
            tc.tile_pool(name="ppt", bufs=1, space="PSUM") as ppt,
            tc.tile_pool(name="pps", bufs=3, space="PSUM") as pps,
            tc.tile_pool(name="ppv", bufs=1, space="PSUM") as ppv,
        ):
            _pref = {}

            def fetch_blk(blk, ht_eng=None):
                ht = hp.tile([128, KC * TB], bf16, tag="ht", name=f"ht{blk}")
                (ht_eng or nc.sync).dma_start(ht, hT_d[blk])
                a_t = rp.tile([128, 2 * 192], bf16, tag="a", name=f"a{blk}")
                nc.scalar.dma_start(a_t, A_d[blk])
                b_t = rp.tile([128, 2 * 192], bf16, tag="b", name=f"b{blk}")
                nc.scalar.dma_start(b_t, B_d[blk])
                return ht, a_t, b_t

            # block 0 rides the scalar queue while the weight groups stream
            # on sync, so the first chain's inputs arrive in parallel
            w_grp = []

            def fetch_wgrp(gi, eng):
                t = singles.tile([128, 3 * PK], bf16, tag=f"w{gi}",
                                 name=f"w{gi}")
                eng.dma_start(t, wall_d[:, gi * 3 * PK:(gi + 1) * 3 * PK])
                w_grp.append(t.rearrange("p (k c) -> p k c", k=3))

            fetch_wgrp(0, nc.sync)
            fetch_wgrp(1, nc.scalar)
            _pref[0] = fetch_blk(0)
            fetch_wgrp(2, nc.sync)
            fetch_wgrp(3, nc.scalar)
            fetch_wgrp(4, nc.sync)
            fetch_wgrp(5, nc.scalar)

            ident = singles.tile([128, 128], bf16, tag="ident", name="ident")
            make_identity(nc, ident)
            bvi_sb = singles.tile([128, HPC * (HD + 1)], f32, tag="bvi",
                                  name="bvi_sb")
            nc.gpsimd.dma_start(out=bvi_sb,
                                in_=bvi_d.to_broadcast([128, HPC * (HD + 1)]))
            if biased:
                bqk_sb = singles.tile([128, HPC * 192], f32, tag="bqk",
                                      name="bqk_sb")
                nc.gpsimd.dma_start(out=bqk_sb,
                                    in_=bqk_d.to_broadcast([128, HPC * 192]))

            qT, kT, vt, ots = {}, {}, {}, {}
            pending = []       # attention instances awaiting emission
            prev_sub = []      # sub-tiles whose transposes are not yet emitted
            ndma = [0]

            def rope_pair(src, a_s, b_s, qrp_slice):
                """qr = src*A + rot_half(src)*B for one head's [q|k] pair."""
                sv = src.rearrange("p (d c u f) -> p d c u f", d=2, c=3, u=2)
                ov = qrp_slice.rearrange("p (d c u f) -> p d c u f",
                                         d=2, c=3, u=2)
                av = a_s
                bv = b_s.rearrange("p (d c u f) -> p d c u f", d=2, c=3, u=2)
                nc.vector.tensor_tensor(ov[:, :, :, 0:1, :],
                                        sv[:, :, :, 1:2, :],
                                        bv[:, :, :, 0:1, :], MUL)
                nc.vector.tensor_tensor(ov[:, :, :, 1:2, :],
                                        sv[:, :, :, 0:1, :],
                                        bv[:, :, :, 1:2, :], MUL)
                t2 = tmpp.tile([128, 192], f32, tag="t2", name="t2")
                nc.vector.tensor_tensor(t2, src, av, MUL)
                nc.vector.tensor_tensor(qrp_slice, t2, qrp_slice, ADD)

            def emit_transposes(sub):
                g, col, qr_t = sub
                pt = ppt.tile([HD, 6 * 128], bf16, tag="pt", name="pt")
                for h in range(HPC):
                    nc.tensor.transpose(
                        pt[:, h * 128:(h + 1) * 128],
                        qr_t[:, h * 192:h * 192 + 96], ident)
                for h in range(HPC):
                    nc.tensor.transpose(
                        pt[:, (3 + h) * 128:(4 + h) * 128],
                        qr_t[:, h * 192 + 96:(h + 1) * 192], ident)
                nc.scalar.copy(
                    qT[g].rearrange("d (h t) -> d h t", h=HPC)
                    [:, :, col:col + 128],
                    pt[:, 0:384].rearrange("d (h t) -> d h t", h=HPC))
                nc.scalar.copy(
                    kT[g].rearrange("d (h t) -> d h t", h=HPC)
                    [:, :, col:col + 128],
                    pt[:, 384:768].rearrange("d (h t) -> d h t", h=HPC))

            def attn_steps(g, h, qh):
                """scoresT -> exp -> PV for one (group, head, query-half) as
                a list of emission thunks, to be interleaved between QKV
                chain matmuls so the exp handoffs hide behind them."""
                pv = ppv.tile([HD + 1, 512], f32, tag="pv",
                              name=f"pv{g}_{h}_{qh}")
                qs = qT[g][:, h * G + qh * 512:h * G + (qh + 1) * 512]
                sts = {}

                def mk_st(kc):
                    def f():
                        st = pps.tile([128, 512], f32, tag="st",
                                      name=f"st{g}_{h}_{qh}_{kc}")
                        nc.tensor.matmul(
                            st,
                            kT[g][:, h * G + kc * 128:h * G + (kc + 1) * 128],
                            qs, start=True, stop=True)
                        sts[kc] = st
                    return f

                def mk_exp_pv(kc):
                    def f():
                        ex = ep.tile([128, 512], bf16, tag="ex",
                                     name=f"ex{g}_{h}_{qh}_{kc}")
                        nc.scalar.activation(ex, sts[kc], Exp, scale=SCALE)
                        nc.tensor.matmul(
                            pv, vt[(g, kc)][:, h * 97:(h + 1) * 97], ex,
                            start=(kc == 0), stop=(kc == 7))
                    return f

                def fin():
                    ot = op.tile([HD + 1, 512], bf16, tag="ot",
                                 name=f"ot{g}_{h}_{qh}")
                    nc.vector.tensor_copy(ot, pv)
                    eng = nc.sync if ndma[0] % 2 == 0 else nc.scalar
                    ndma[0] += 1
                    eng.dma_start(outN_d[h, g * 2 + qh], ot)

                steps = [mk_st(0), mk_st(1), mk_st(2)]
                for kc in range(3, 8):
                    steps.append(mk_exp_pv(kc - 3))
                    steps.append(mk_st(kc))
                steps.extend([mk_exp_pv(kc) for kc in range(5, 8)])
                steps.append(fin)
                return steps

            def attn_instance(g, h, qh):
                for f in attn_steps(g, h, qh):
                    f()

            for blk in range(NB):
                g = blk // 4
                if blk % 4 == 0:
                    qT[g] = qtp.tile([HD, HPC * G], bf16, tag="qT",
                                     name=f"qT{g}")
                    kT[g] = ktp.tile([HD, HPC * G], bf16, tag="kT",
                                     name=f"kT{g}")
                ht, a_t, b_t = _pref.pop(blk) if blk in _pref else fetch_blk(blk)
                if blk + 1 < NB:
                    _pref[blk + 1] = fetch_blk(blk + 1)
                htv = ht.rearrange("p (k t) -> p k t", k=KC)

                for sub in range(2):
                    tb = blk * 2 + sub
                    col = (tb % 8) * 128
                    # packed QKV as two sequential accumulation chains, so
                    # the second chain's psum slot (reused from the previous
                    # sub-tile) has a full chain's time to be consumed
                    ps1 = ppq.tile([128, 480], f32, tag="ps", name=f"ps1_{tb}")
                    ps2 = ppq.tile([128, 384], f32, tag="ps", name=f"ps2_{tb}")
                    for kc in range(KC):
                        nc.tensor.matmul(ps1,
                                         htv[:, kc, sub * 128:(sub + 1) * 128],
                                         w_grp[kc // 3][:, kc % 3, 0:480],
                                         start=(kc == 0), stop=(kc == KC - 1))
                    for kc in range(KC):
                        nc.tensor.matmul(ps2,
                                         htv[:, kc, sub * 128:(sub + 1) * 128],
                                         w_grp[kc // 3][:, kc % 3, 480:PK],
                                         start=(kc == 0), stop=(kc == KC - 1))
                    # rope on the three contiguous [q|k] psum pairs
                    qr_t = qrp.tile([128, HPC * 192], bf16, tag="qr",
                                    name=f"qr{tb}")
                    pairs = (ps1[:, 0:192], ps1[:, 192:384], ps2[:, 0:192])
                    a_s = a_t[:, sub * 192:(sub + 1) * 192]
                    b_s = b_t[:, sub * 192:(sub + 1) * 192]
                    for h, pr in enumerate(pairs):
                        src = pr
                        if biased:
                            stg = tmpp.tile([128, 192], f32, tag="stg",
                                            name=f"stg{tb}_{h}")
                            nc.vector.tensor_tensor(
                                stg, pr, bqk_sb[:, h * 192:(h + 1) * 192], ADD)
                            src = stg
                        rope_pair(src, a_s, b_s,
                                  qr_t[:, h * 192:(h + 1) * 192])
                    # V: bias add + ones column (Pool engine), cast to bf16
                    v_t = vp.tile([128, HPC * (HD + 1)], bf16, tag="v",
                                  name=f"v{tb}")
                    v3 = v_t.rearrange("p (h c) -> p h c", h=HPC)
                    bv3 = bvi_sb.rearrange("p (h c) -> p h c", h=HPC)
                    nc.vector.tensor_tensor(
                        v3[:, 0:1, 0:96], ps1[:, 384:480].rearrange(
                            "p (h c) -> p h c", h=1), bv3[:, 0:1, 0:96], ADD)
                    nc.vector.tensor_tensor(
                        v3[:, 1:3, 0:96], ps2[:, 192:384].rearrange(
                            "p (h c) -> p h c", h=2), bv3[:, 1:3, 0:96], ADD)
                    nc.gpsimd.memset(v3[:, :, 96:97], 1.0)
                    vt[(g, tb % 8)] = v_t
                    # transposes for the PREVIOUS sub-tile (deps long ready)
                    prev_sub.append((g, col, qr_t))
                    if len(prev_sub) > 1:
                        emit_transposes(prev_sub.pop(0))
                    # drain one pending attention instance per sub-tile
                    if pending:
                        attn_instance(*pending.pop(0))
                if blk % 4 == 3:
                    gq = blk // 4
                    pending.extend((gq, h, qh)
                                   for h in range(HPC) for qh in range(2))
            while prev_sub:
                emit_transposes(prev_sub.pop(0))
            while pending:
                attn_instance(*pending.pop(0))
    nc.compile()
    return nc


def _build_launch2(biased):
    import concourse.mybir as mybir
    import concourse.tile as tile
    from concourse import bacc

    f32 = mybir.dt.float32
    bf16 = mybir.dt.bfloat16
    TOK = 1024           # tokens per core
    NW = 1152            # outdims per core
    MB = NW // 128       # 9 outdim blocks
    nc = bacc.Bacc("TRN2", target_bir_lowering=False, debug=False)

    at_d = nc.dram_tensor("attnT", [DIM, TOK], bf16, kind="ExternalInput").ap()
    wo_d = nc.dram_tensor("woj", [DIM, NW], bf16, kind="ExternalInput").ap()
    if biased:
        bo_d = nc.dram_tensor("boj", [1, NW], f32, kind="ExternalInput").ap()
    # transposed output [outdim, tok]; host transposes back
    out_d = nc.dram_tensor("out", [NW, TOK], bf16, kind="ExternalOutput").ap()

    with tile.TileContext(nc) as tc:
        ats, wos = [], []
        with (
            tc.tile_pool(name="singles2", bufs=1) as singles,
            tc.tile_pool(name="atp", bufs=KC) as atp,
            tc.tile_pool(name="wop", bufs=KC) as wop,
            tc.tile_pool(name="outp", bufs=3) as outp,
            tc.tile_pool(name="psp", bufs=8, space="PSUM") as psp,
        ):
            # at chunks on one HWDGE queue, wo chunks on the other
            for kc in range(KC):
                a = atp.tile([128, TOK], bf16, tag="at", name=f"at{kc}")
                nc.sync.dma_start(a, at_d[kc * 128:(kc + 1) * 128, :])
                ats.append(a)
                w = wop.tile([128, NW], bf16, tag="wo", name=f"wo{kc}")
                nc.scalar.dma_start(w, wo_d[kc * 128:(kc + 1) * 128, :])
                wos.append(w)
            if biased:
                bo_sb = singles.tile([128, MB], f32, tag="bo_sb", name="bo_sb")
                nc.sync.dma_start(bo_sb,
                                  bo_d.rearrange("a (m p) -> p (a m)", p=128))
            # kc-outer over groups of 8 units: the first group has enough
            # matmul work per chunk to stay ahead of the (bandwidth-bound)
            # 10MB input sweep, and a unit only needs chunk kc when every
            # unit does
            units = [(mb, th) for mb in range(MB) for th in range(2)]
            ot_tiles = {}
            for base in range(0, len(units), 8):
                grp = units[base:base + 8]
                pss = {u: psp.tile([128, 512], f32, tag="ps",
                                   name=f"ps{u[0]}_{u[1]}") for u in grp}
                for kc in range(KC):
                    for mb, th in grp:
                        nc.tensor.matmul(
                            pss[(mb, th)], wos[kc][:, mb * 128:(mb + 1) * 128],
                            ats[kc][:, th * 512:(th + 1) * 512],
                            start=(kc == 0), stop=(kc == KC - 1))
                for mb, th in grp:
                    if mb not in ot_tiles:
                        ot_tiles[mb] = outp.tile([128, TOK], bf16, tag="ot",
                                                 name=f"ot{mb}")
                    dst = ot_tiles[mb][:, th * 512:(th + 1) * 512]
                    if biased:
                        nc.vector.tensor_scalar_add(dst, pss[(mb, th)],
                                                    bo_sb[:, mb:mb + 1])
                    else:
                        nc.vector.tensor_copy(dst, pss[(mb, th)])
                    if th == 1:
                        eng = nc.sync if mb % 2 == 0 else nc.scalar
                        eng.dma_start(out_d[mb * 128:(mb + 1) * 128, :],
                                      ot_tiles[mb])
    nc.compile()
    return nc


def _get(name, builder, *args):
    if name not in _CACHE:
        _CACHE[name] = builder(*args)
    return _CACHE[name]


def _rope_tables(frame, height, width):
    t = np.repeat(np.arange(frame), height * width)
    y = np.tile(np.repeat(np.arange(height), width), frame)
    x = np.tile(np.arange(width), frame * height)
    D = HD // 3
    A = np.empty((S, HD), np.float32)
    B = np.empty((S, HD), np.float32)
    for i, pos in enumerate((t, y, x)):
        inv = 1.0 / (10000.0 ** (np.arange(0, D, 2, dtype=np.float32) / D))
        f = pos[:, None].astype(np.float32) * inv[None, :]
        A[:, i * D:i * D + 16] = np.cos(f)
        A[:, i * D + 16:(i + 1) * D] = np.cos(f)
        B[:, i * D:i * D + 16] = -np.sin(f)
        B[:, i * D + 16:(i + 1) * D] = np.sin(f)
    return A, B


def _tile_hT(hT, bf):
    # [2304, 4096] -> [NB, 128, KC*TB]: blk-major, partition-major, then
    # (chunk, token) contiguous per partition
    return np.ascontiguousarray(
        hT.reshape(KC, 128, NB, TB).transpose(2, 1, 0, 3).reshape(
            NB, 128, KC * TB).astype(bf))


def _tile_rope(a, bf):
    # [4096, 192] (qk-pair dup) -> [NB, 128, 2*192]
    return np.ascontiguousarray(
        a.reshape(NB, 2, 128, 192).transpose(0, 2, 1, 3).reshape(
            NB, 128, 2 * 192).astype(bf))


def _pack_wall(wq, wk, wv, sl, bf):
    # per-core packed qkv weight [2304, 864]:
    #   per chunk columns seg1=[q0|k0|q1|k1|v0] seg2=[q2|k2|v1|v2]
    q, k, v = wq[:, sl], wk[:, sl], wv[:, sl]
    h = [slice(i * 96, (i + 1) * 96) for i in range(3)]
    wall = np.concatenate(
        [q[:, h[0]], k[:, h[0]], q[:, h[1]], k[:, h[1]], v[:, h[0]],
         q[:, h[2]], k[:, h[2]], v[:, h[1]], v[:, h[2]]], axis=1)
    return np.ascontiguousarray(
        wall.reshape(KC, 128, PK).transpose(1, 0, 2).reshape(
            128, KC * PK).astype(bf))


def kernel(hidden_states, wq, bq, wk, bk, wv, bv, wo, bo, frame, height, width):
    import ml_dtypes
    from concourse import bass_utils

    bf = ml_dtypes.bfloat16
    f, hh, ww = int(frame), int(height), int(width)
    hs = np.asarray(hidden_states, dtype=np.float32)
    assert hs.shape == (1, S, DIM) and f * hh * ww == S
    wq, wk, wv, wo = (np.asarray(a, np.float32) for a in (wq, wk, wv, wo))
    bq, bk, bv, bo = (np.asarray(a, np.float32) for a in (bq, bk, bv, bo))
    biased = bool(bq.any() or bk.any())

    perm = np.concatenate([np.arange(k, S, SPN) for k in range(SPN)])
    A, B = _rope_tables(f, hh, ww)
    A = _tile_rope(np.tile(A[perm], (1, 2)), bf)
    B = _tile_rope(np.tile(B[perm], (1, 2)), bf)
    hT = _tile_hT(hs[0].T[:, perm], bf)

    nc1 = _get(f"l1_{biased}", _build_launch1, biased)
    in1 = []
    for c in range(8):
        sl = slice(c * CW, (c + 1) * CW)
        d = {
            "hT": hT,
            "wall": _pack_wall(wq, wk, wv, sl, bf),
            "bvi": np.concatenate(
                [np.concatenate([bv[sl][h * HD:(h + 1) * HD], [0.0]])
                 for h in range(HPC)]).astype(np.float32).reshape(1, -1),
            "A": A, "B": B,
        }
        if biased:
            d["bqk"] = np.concatenate(
                [np.concatenate([bq[sl][h * HD:(h + 1) * HD],
                                 bk[sl][h * HD:(h + 1) * HD]])
                 for h in range(HPC)]).astype(np.float32).reshape(1, -1)
        in1.append(d)
    td = os.environ.get("KERNEL_TRACE_DIR")
    if td:
        os.makedirs(td + "/l1", exist_ok=True)
        for fn in os.listdir(td + "/l1"):
            os.unlink(td + "/l1/" + fn)
    res1 = bass_utils.run_bass_kernel_spmd(
        nc1, in1, core_ids=list(range(8)),
        tmpdir=(td + "/l1") if td else None)
    LAST_RESULTS.append(res1)

    # outN [HPC, 8, 97, 512] bf16 -> [HPC*97, 4096] fp32, normalize, unpermute
    outN = np.concatenate(
        [np.asarray(res1.results[c]["outN"]).astype(np.float32)
         .transpose(0, 2, 1, 3).reshape(HPC, HD + 1, S) for c in range(8)], 0
    ).reshape(8 * HPC, HD + 1, S)
    attnT_g = (outN[:, :HD, :] / outN[:, HD:HD + 1, :]).reshape(DIM, S)
    attnT = np.empty_like(attnT_g)
    attnT[:, perm] = attnT_g

    biased2 = bool(bo.any())
    nc2 = _get(f"l2_{biased2}", _build_launch2, biased2)
    in2 = []
    for c in range(8):
        i, j = divmod(c, 2)
        d2 = {
            "attnT": np.ascontiguousarray(
                attnT[:, i * 1024:(i + 1) * 1024].astype(bf)),
            "woj": np.ascontiguousarray(
                wo[:, j * 1152:(j + 1) * 1152].astype(bf)),
        }
        if biased2:
            d2["boj"] = np.ascontiguousarray(
                bo[j * 1152:(j + 1) * 1152]).reshape(1, 1152)
        in2.append(d2)
    if td:
        os.makedirs(td + "/l2", exist_ok=True)
        for fn in os.listdir(td + "/l2"):
            os.unlink(td + "/l2/" + fn)
    res2 = bass_utils.run_bass_kernel_spmd(
        nc2, in2, core_ids=list(range(8)),
        tmpdir=(td + "/l2") if td else None)
    LAST_RESULTS.append(res2)

    out = np.empty((S, DIM), np.float32)
    for c in range(8):
        i, j = divmod(c, 2)
        out[i * 1024:(i + 1) * 1024, j * 1152:(j + 1) * 1152] = \
            np.asarray(res2.results[c]["out"]).T.astype(np.float32)
    return out[None]


# revision 31
# speedup vs baseline: 1.0225x; 1.0205x over previous
"""Trainium2 Bass kernel for nn_BasicTransformerBlock_18657337934637.

Sparse-attention transformer block:
  q/k/v = hidden @ W* + b*        (2304 -> 2304, 24 heads x 96)
  RoPE3D on q, k
  sparse-1d grouping (SPARSE_N=4): token t -> group t%4, 1024 tokens/group
  softmax attention within each (group, head)
  out = attn @ wo + bo

Distribution over 8 NeuronCores:
  Launch 1 (head-parallel): core c computes heads 3c..3c+2 end-to-end through
    attention.  Host pre-transposes hidden to hT [2304, 4096] in grouped token
    order (groups = contiguous 1024-token spans).  All matmul streams are
    bf16 (fp32 PSUM accumulation); rel tolerance is 2e-2 and bf16 keeps the
    full pipeline well under 1e-2.
    QKV: per 128-token sub-tile the three projections are packed into TWO
    moving-weight matmuls per contraction chunk (480+384 columns) instead of
    three 288-col ones.  Packed column layout per chunk:
      seg1[480] = [q0|k0|q1|k1|v0]   seg2[384] = [q2|k2|v1|v2]
    so each head's (q|k) pair is contiguous for rope, which reads PSUM
    directly (no staging copy).  Per (group, head): scores computed
    transposed [k, q]; exp on the scalar engine; an all-ones column in v
    yields the softmax denominator in the same PV matmul.  PSUM->SBUF
    copies run on the (otherwise idle) Pool engine.  Output: un-normalized
    attn^T + denominator row as contiguous bf16 [97, 512] blocks; the host
    divides.
  Host: normalize, gather heads -> attnT [2304, 4096], undo permutation.
  Launch 2 (token x outdim parallel): core (i, j) computes
    out[i*1024:(i+1)*1024, j*1152:(j+1)*1152]^T = wo_j^T @ attnT_i in bf16,
    as 9 rolling psum unit-pairs (no group barriers), inputs split across
    both HWDGE queues.
"""
import os
import numpy as np

HEADS = 24
HD = 96
SPN = 4
S = 4096
DIM = 2304
KC = DIM // 128            # 18 contraction chunks
HPC = 3                    # heads per core
CW = HPC * HD              # 288 projection columns per core
PK = 864                   # packed qkv columns per chunk (3*288)
G = S // SPN               # 1024 tokens per group
TB = 256                   # hT dma block (tokens)
NB = S // TB               # 16 blocks
WG = 6                     # weight dma chunk-groups (3 kc each)
SCALE = 1.0 / float(np.sqrt(HD))

_CACHE = {}
LAST_RESULTS = []          # test harness introspection


def _build_launch1(biased):
    import concourse.mybir as mybir
    import concourse.tile as tile
    from concourse import bacc
    from concourse.masks import make_identity

    f32 = mybir.dt.float32
    bf16 = mybir.dt.bfloat16
    Exp = mybir.ActivationFunctionType.Exp
    MUL = mybir.AluOpType.mult
    ADD = mybir.AluOpType.add
    nc = bacc.Bacc("TRN2", target_bir_lowering=False, debug=False)

    # all inputs host-pre-tiled to the exact SBUF layouts -> every DMA is a
    # plain 2D copy with multi-KB contiguous rows
    hT_d = nc.dram_tensor("hT", [NB, 128, KC * TB], bf16,
                          kind="ExternalInput").ap()
    wall1_d = nc.dram_tensor("wall1", [128, KC * 480], bf16,
                             kind="ExternalInput").ap()
    wall2_d = nc.dram_tensor("wall2", [128, KC * 384], bf16,
                             kind="ExternalInput").ap()
    A_d = nc.dram_tensor("A", [NB, 128, 2 * 192], bf16,
                         kind="ExternalInput").ap()
    B_d = nc.dram_tensor("B", [NB, 128, 2 * 192], bf16,
                         kind="ExternalInput").ap()
    bvi_d = nc.dram_tensor("bvi", [1, HPC * (HD + 1)], f32,
                           kind="ExternalInput").ap()
    if biased:
        bqk_d = nc.dram_tensor("bqk", [1, HPC * 192], f32,
                               kind="ExternalInput").ap()
    outN_d = nc.dram_tensor("outN", [HPC, 2 * SPN, HD + 1, 512], bf16,
                            kind="ExternalOutput").ap()

    with tile.TileContext(nc) as tc:
        with (
            tc.tile_pool(name="singles", bufs=1) as singles,
            tc.tile_pool(name="hp", bufs=2) as hp,
            tc.tile_pool(name="rp", bufs=3) as rp,
            tc.tile_pool(name="qrp", bufs=3) as qrp,
            tc.tile_pool(name="tmp", bufs=2) as tmpp,
            tc.tile_pool(name="vp", bufs=16) as vp,
            tc.tile_pool(name="qtp", bufs=2) as qtp,
            tc.tile_pool(name="ktp", bufs=2) as ktp,
            tc.tile_pool(name="ep", bufs=3) as ep,
            tc.tile_pool(name="op", bufs=6) as op,
            tc.tile_pool(name="ppq", bufs=3, space="PSUM") as ppq,
            tc.tile_pool(name="ppt", bufs=1, space="PSUM") as ppt,
            tc.tile_pool(name="pps", bufs=3, space="PSUM") as pps,
            tc.tile_pool(name="ppv", bufs=1, space="PSUM") as ppv,
        ):
            _pref = {}
            HHALF = 9 * TB     # ht column split (chunks 0-8 / 9-17)

            def fetch_blk(blk):
                # each block's hT rides both HWDGE queues as two halves
                ht = hp.tile([128, KC * TB], bf16, tag="ht", name=f"ht{blk}")
                nc.sync.dma_start(ht[:, 0:HHALF], hT_d[blk][:, 0:HHALF])
                nc.scalar.dma_start(ht[:, HHALF:], hT_d[blk][:, HHALF:])
                a_t = rp.tile([128, 2 * 192], bf16, tag="a", name=f"a{blk}")
                nc.scalar.dma_start(a_t, A_d[blk])
                b_t = rp.tile([128, 2 * 192], bf16, tag="b", name=f"b{blk}")
                nc.scalar.dma_start(b_t, B_d[blk])
                return ht, a_t, b_t

            # seg1 weights + block 0 first: chain1's full input set (1.11MB
            # + 1.18MB) lands ~14us in, so projection matmuls start early;
            # seg2 weights stream in behind while chain1s run
            w1_grp, w2_grp = [], []

            def fetch_wgrp(grps, dram, w, gi, eng):
                t = singles.tile([128, 3 * w], bf16, tag=f"w{len(grps)}_{w}",
                                 name=f"w{gi}_{w}")
                eng.dma_start(t, dram[:, gi * 3 * w:(gi + 1) * 3 * w])
                grps.append(t.rearrange("p (k c) -> p k c", k=3))

            fetch_wgrp(w1_grp, wall1_d, 480, 0, nc.sync)
            fetch_wgrp(w1_grp, wall1_d, 480, 1, nc.scalar)
            ht0 = hp.tile([128, KC * TB], bf16, tag="ht", name="ht0")
            nc.sync.dma_start(ht0[:, 0:HHALF], hT_d[0][:, 0:HHALF])
            nc.scalar.dma_start(ht0[:, HHALF:], hT_d[0][:, HHALF:])
            fetch_wgrp(w1_grp, wall1_d, 480, 2, nc.sync)
            fetch_wgrp(w1_grp, wall1_d, 480, 3, nc.scalar)
            fetch_wgrp(w1_grp, wall1_d, 480, 4, nc.sync)
            fetch_wgrp(w1_grp, wall1_d, 480, 5, nc.scalar)
            a0 = rp.tile([128, 2 * 192], bf16, tag="a", name="a0")
            nc.scalar.dma_start(a0, A_d[0])
            b0 = rp.tile([128, 2 * 192], bf16, tag="b", name="b0")
            nc.scalar.dma_start(b0, B_d[0])
            _pref[0] = (ht0, a0, b0)
            for gi in range(WG):
                fetch_wgrp(w2_grp, wall2_d, 384, gi,
                           nc.sync if gi % 2 == 0 else nc.scalar)

            ident = singles.tile([128, 128], bf16, tag="ident", name="ident")
            make_identity(nc, ident)
            bvi_sb = singles.tile([128, HPC * (HD + 1)], f32, tag="bvi",
                                  name="bvi_sb")
            nc.gpsimd.dma_start(out=bvi_sb,
                                in_=bvi_d.to_broadcast([128, HPC * (HD + 1)]))
            if biased:
                bqk_sb = singles.tile([128, HPC * 192], f32, tag="bqk",
                                      name="bqk_sb")
                nc.gpsimd.dma_start(out=bqk_sb,
                                    in_=bqk_d.to_broadcast([128, HPC * 192]))

            qT, kT, vt, ots = {}, {}, {}, {}
            pending = []       # attention instances awaiting emission
            prev_sub = []      # sub-tiles whose transposes are not yet emitted
            ndma = [0]

            def rope_pair(src, a_s, b_s, qrp_slice):
                """qr = src*A + rot_half(src)*B for one head's [q|k] pair."""
                sv = src.rearrange("p (d c u f) -> p d c u f", d=2, c=3, u=2)
                ov = qrp_slice.rearrange("p (d c u f) -> p d c u f",
                                         d=2, c=3, u=2)
                av = a_s
                bv = b_s.rearrange("p (d c u f) -> p d c u f", d=2, c=3, u=2)
                nc.vector.tensor_tensor(ov[:, :, :, 0:1, :],
                                        sv[:, :, :, 1:2, :],
                                        bv[:, :, :, 0:1, :], MUL)
                nc.vector.tensor_tensor(ov[:, :, :, 1:2, :],
                                        sv[:, :, :, 0:1, :],
                                        bv[:, :, :, 1:2, :], MUL)
                t2 = tmpp.tile([128, 192], f32, tag="t2", name="t2")
                nc.vector.tensor_tensor(t2, src, av, MUL)
                nc.vector.tensor_tensor(qrp_slice, t2, qrp_slice, ADD)

            def emit_transposes(sub):
                g, col, qr_t = sub
                pt = ppt.tile([HD, 6 * 128], bf16, tag="pt", name="pt")
                for h in range(HPC):
                    nc.tensor.transpose(
                        pt[:, h * 128:(h + 1) * 128],
                        qr_t[:, h * 192:h * 192 + 96], ident)
                for h in range(HPC):
                    nc.tensor.transpose(
                        pt[:, (3 + h) * 128:(4 + h) * 128],
                        qr_t[:, h * 192 + 96:(h + 1) * 192], ident)
                nc.scalar.copy(
                    qT[g].rearrange("d (h t) -> d h t", h=HPC)
                    [:, :, col:col + 128],
                    pt[:, 0:384].rearrange("d (h t) -> d h t", h=HPC))
                nc.scalar.copy(
                    kT[g].rearrange("d (h t) -> d h t", h=HPC)
                    [:, :, col:col + 128],
                    pt[:, 384:768].rearrange("d (h t) -> d h t", h=HPC))

            def attn_steps(g, h, qh):
                """scoresT -> exp -> PV for one (group, head, query-half) as
                a list of emission thunks, to be interleaved between QKV
                chain matmuls so the exp handoffs hide behind them."""
                pv = ppv.tile([HD + 1, 512], f32, tag="pv",
                              name=f"pv{g}_{h}_{qh}")
                qs = qT[g][:, h * G + qh * 512:h * G + (qh + 1) * 512]
                sts = {}

                def mk_st(kc):
                    def f():
                        st = pps.tile([128, 512], f32, tag="st",
                                      name=f"st{g}_{h}_{qh}_{kc}")
                        nc.tensor.matmul(
                            st,
                            kT[g][:, h * G + kc * 128:h * G + (kc + 1) * 128],
                            qs, start=True, stop=True)
                        sts[kc] = st
                    return f

                def mk_exp_pv(kc):
                    def f():
                        ex = ep.tile([128, 512], bf16, tag="ex",
                                     name=f"ex{g}_{h}_{qh}_{kc}")
                        nc.scalar.activation(ex, sts[kc], Exp, scale=SCALE)
                        nc.tensor.matmul(
                            pv, vt[(g, kc)][:, h * 97:(h + 1) * 97], ex,
                            start=(kc == 0), stop=(kc == 7))
                    return f

                def fin():
                    ot = op.tile([HD + 1, 512], bf16, tag="ot",
                                 name=f"ot{g}_{h}_{qh}")
                    nc.vector.tensor_copy(ot, pv)
                    eng = nc.sync if ndma[0] % 2 == 0 else nc.scalar
                    ndma[0] += 1
                    eng.dma_start(outN_d[h, g * 2 + qh], ot)

                steps = [mk_st(0), mk_st(1), mk_st(2)]
                for kc in range(3, 8):
                    steps.append(mk_exp_pv(kc - 3))
                    steps.append(mk_st(kc))
                steps.extend([mk_exp_pv(kc) for kc in range(5, 8)])
                steps.append(fin)
                return steps

            def attn_instance(g, h, qh):
                for f in attn_steps(g, h, qh):
                    f()

            def emit_chain(htv, sub, ps, grp):
                for kc in range(KC):
                    nc.tensor.matmul(ps,
                                     htv[:, kc, sub * 128:(sub + 1) * 128],
                                     grp[kc // 3][:, kc % 3, :],
                                     start=(kc == 0), stop=(kc == KC - 1))

            def do_pair(tb, h, pr, a_s, b_s, qr_t):
                src = pr
                if biased:
                    stg = tmpp.tile([128, 192], f32, tag="stg",
                                    name=f"stg{tb}_{h}")
                    nc.vector.tensor_tensor(
                        stg, pr, bqk_sb[:, h * 192:(h + 1) * 192], ADD)
                    src = stg
                rope_pair(src, a_s, b_s, qr_t[:, h * 192:(h + 1) * 192])

            def post_sub(g, tb, sub, ps1, ps2, a_t, b_t):
                col = (tb % 8) * 128
                qr_t = qrp.tile([128, HPC * 192], bf16, tag="qr",
                                name=f"qr{tb}")
                a_s = a_t[:, sub * 192:(sub + 1) * 192]
                b_s = b_t[:, sub * 192:(sub + 1) * 192]
                v_t = vp.tile([128, HPC * (HD + 1)], bf16, tag="v",
                              name=f"v{tb}")
                v3 = v_t.rearrange("p (h c) -> p h c", h=HPC)
                bv3 = bvi_sb.rearrange("p (h c) -> p h c", h=HPC)
                # ps1 readers first (pairs 0,1 + v0) so its psum slot frees
                # as soon as possible after chain1's stop
                do_pair(tb, 0, ps1[:, 0:192], a_s, b_s, qr_t)
                do_pair(tb, 1, ps1[:, 192:384], a_s, b_s, qr_t)
                nc.vector.tensor_tensor(
                    v3[:, 0:1, 0:96], ps1[:, 384:480].rearrange(
                        "p (h c) -> p h c", h=1), bv3[:, 0:1, 0:96], ADD)
                do_pair(tb, 2, ps2[:, 0:192], a_s, b_s, qr_t)
                nc.vector.tensor_tensor(
                    v3[:, 1:3, 0:96], ps2[:, 192:384].rearrange(
                        "p (h c) -> p h c", h=2), bv3[:, 1:3, 0:96], ADD)
                nc.gpsimd.memset(v3[:, :, 96:97], 1.0)
                vt[(g, tb % 8)] = v_t
                # transposes for the PREVIOUS sub-tile (deps long ready)
                prev_sub.append((g, col, qr_t))
                if len(prev_sub) > 1:
                    emit_transposes(prev_sub.pop(0))
                # drain one pending attention instance per sub-tile
                if pending:
                    attn_instance(*pending.pop(0))

            for blk in range(NB):
                g = blk // 4
                if blk % 4 == 0:
                    qT[g] = qtp.tile([HD, HPC * G], bf16, tag="qT",
                                     name=f"qT{g}")
                    kT[g] = ktp.tile([HD, HPC * G], bf16, tag="kT",
                                     name=f"kT{g}")
                ht, a_t, b_t = _pref.pop(blk) if blk in _pref else fetch_blk(blk)
                if blk + 1 < NB:
                    _pref[blk + 1] = fetch_blk(blk + 1)
                htv = ht.rearrange("p (k t) -> p k t", k=KC)

                if blk == 0:
                    # both seg1 chains first: they only need wall1 + ht0, so
                    # the PE starts ~5us earlier while wall2 streams in
                    ps1a = ppq.tile([128, 480], f32, tag="ps", name="ps1_0")
                    emit_chain(htv, 0, ps1a, w1_grp)
                    ps1b = ppq.tile([128, 480], f32, tag="ps", name="ps1_1")
                    emit_chain(htv, 1, ps1b, w1_grp)
                    ps2a = ppq.tile([128, 384], f32, tag="ps", name="ps2_0")
                    emit_chain(htv, 0, ps2a, w2_grp)
                    post_sub(g, 0, 0, ps1a, ps2a, a_t, b_t)
                    ps2b = ppq.tile([128, 384], f32, tag="ps", name="ps2_1")
                    emit_chain(htv, 1, ps2b, w2_grp)
                    post_sub(g, 1, 1, ps1b, ps2b, a_t, b_t)
                    continue

                for sub in range(2):
                    tb = blk * 2 + sub
                    # packed QKV as two sequential accumulation chains, so
                    # the second chain's psum slot (reused from the previous
                    # sub-tile) has a full chain's time to be consumed
                    ps1 = ppq.tile([128, 480], f32, tag="ps", name=f"ps1_{tb}")
                    emit_chain(htv, sub, ps1, w1_grp)
                    ps2 = ppq.tile([128, 384], f32, tag="ps", name=f"ps2_{tb}")
                    emit_chain(htv, sub, ps2, w2_grp)
                    post_sub(g, tb, sub, ps1, ps2, a_t, b_t)
                if blk % 4 == 3:
                    gq = blk // 4
                    pending.extend((gq, h, qh)
                                   for h in range(HPC) for qh in range(2))
            while prev_sub:
                emit_transposes(prev_sub.pop(0))
            while pending:
                attn_instance(*pending.pop(0))
    nc.compile()
    return nc


def _build_launch2(biased):
    import concourse.mybir as mybir
    import concourse.tile as tile
    from concourse import bacc

    f32 = mybir.dt.float32
    bf16 = mybir.dt.bfloat16
    TOK = 1024           # tokens per core
    NW = 1152            # outdims per core
    MB = NW // 128       # 9 outdim blocks
    nc = bacc.Bacc("TRN2", target_bir_lowering=False, debug=False)

    at_d = nc.dram_tensor("attnT", [DIM, TOK], bf16, kind="ExternalInput").ap()
    wo_d = nc.dram_tensor("woj", [DIM, NW], bf16, kind="ExternalInput").ap()
    if biased:
        bo_d = nc.dram_tensor("boj", [1, NW], f32, kind="ExternalInput").ap()
    # transposed output [outdim, tok]; host transposes back
    out_d = nc.dram_tensor("out", [NW, TOK], bf16, kind="ExternalOutput").ap()

    with tile.TileContext(nc) as tc:
        ats, wos = [], []
        with (
            tc.tile_pool(name="singles2", bufs=1) as singles,
            tc.tile_pool(name="atp", bufs=KC) as atp,
            tc.tile_pool(name="wop", bufs=KC) as wop,
            tc.tile_pool(name="outp", bufs=3) as outp,
            tc.tile_pool(name="psp", bufs=8, space="PSUM") as psp,
        ):
            # at chunks on one HWDGE queue, wo chunks on the other
            for kc in range(KC):
                a = atp.tile([128, TOK], bf16, tag="at", name=f"at{kc}")
                nc.sync.dma_start(a, at_d[kc * 128:(kc + 1) * 128, :])
                ats.append(a)
                w = wop.tile([128, NW], bf16, tag="wo", name=f"wo{kc}")
                nc.scalar.dma_start(w, wo_d[kc * 128:(kc + 1) * 128, :])
                wos.append(w)
            if biased:
                bo_sb = singles.tile([128, MB], f32, tag="bo_sb", name="bo_sb")
                nc.sync.dma_start(bo_sb,
                                  bo_d.rearrange("a (m p) -> p (a m)", p=128))
            # kc-outer over groups of 8 units: the first group has enough
            # matmul work per chunk to stay ahead of the (bandwidth-bound)
            # 10MB input sweep, and a unit only needs chunk kc when every
            # unit does
            units = [(mb, th) for mb in range(MB) for th in range(2)]
            ot_tiles = {}
            for base in range(0, len(units), 8):
                grp = units[base:base + 8]
                pss = {u: psp.tile([128, 512], f32, tag="ps",
                                   name=f"ps{u[0]}_{u[1]}") for u in grp}
                for kc in range(KC):
                    for mb, th in grp:
                        nc.tensor.matmul(
                            pss[(mb, th)], wos[kc][:, mb * 128:(mb + 1) * 128],
                            ats[kc][:, th * 512:(th + 1) * 512],
                            start=(kc == 0), stop=(kc == KC - 1))
                for mb, th in grp:
                    if mb not in ot_tiles:
                        ot_tiles[mb] = outp.tile([128, TOK], bf16, tag="ot",
                                                 name=f"ot{mb}")
                    dst = ot_tiles[mb][:, th * 512:(th + 1) * 512]
                    if biased:
                        nc.vector.tensor_scalar_add(dst, pss[(mb, th)],
                                                    bo_sb[:, mb:mb + 1])
                    else:
                        nc.vector.tensor_copy(dst, pss[(mb, th)])
                    if th == 1:
                        eng = nc.sync if mb % 2 == 0 else nc.scalar
                        eng.dma_start(out_d[mb * 128:(mb + 1) * 128, :],
                                      ot_tiles[mb])
    nc.compile()
    return nc


def _get(name, builder, *args):
    if name not in _CACHE:
        _CACHE[name] = builder(*args)
    return _CACHE[name]


def _rope_tables(frame, height, width):
    t = np.repeat(np.arange(frame), height * width)
    y = np.tile(np.repeat(np.arange(height), width), frame)
    x = np.tile(np.arange(width), frame * height)
    D = HD // 3
    A = np.empty((S, HD), np.float32)
    B = np.empty((S, HD), np.float32)
    for i, pos in enumerate((t, y, x)):
        inv = 1.0 / (10000.0 ** (np.arange(0, D, 2, dtype=np.float32) / D))
        f = pos[:, None].astype(np.float32) * inv[None, :]
        A[:, i * D:i * D + 16] = np.cos(f)
        A[:, i * D + 16:(i + 1) * D] = np.cos(f)
        B[:, i * D:i * D + 16] = -np.sin(f)
        B[:, i * D + 16:(i + 1) * D] = np.sin(f)
    return A, B


def _tile_hT(hT, bf):
    # [2304, 4096] -> [NB, 128, KC*TB]: blk-major, partition-major, then
    # (chunk, token) contiguous per partition
    return np.ascontiguousarray(
        hT.reshape(KC, 128, NB, TB).transpose(2, 1, 0, 3).reshape(
            NB, 128, KC * TB).astype(bf))


def _tile_rope(a, bf):
    # [4096, 192] (qk-pair dup) -> [NB, 128, 2*192]
    return np.ascontiguousarray(
        a.reshape(NB, 2, 128, 192).transpose(0, 2, 1, 3).reshape(
            NB, 128, 2 * 192).astype(bf))


def _pack_wall(wq, wk, wv, sl, bf):
    # per-core packed qkv weights, split by chain so chain1's set is small:
    #   seg1=[q0|k0|q1|k1|v0] (480)   seg2=[q2|k2|v1|v2] (384)
    q, k, v = wq[:, sl], wk[:, sl], wv[:, sl]
    h = [slice(i * 96, (i + 1) * 96) for i in range(3)]
    seg1 = np.concatenate(
        [q[:, h[0]], k[:, h[0]], q[:, h[1]], k[:, h[1]], v[:, h[0]]], axis=1)
    seg2 = np.concatenate(
        [q[:, h[2]], k[:, h[2]], v[:, h[1]], v[:, h[2]]], axis=1)
    tile = lambda w, n: np.ascontiguousarray(
        w.reshape(KC, 128, n).transpose(1, 0, 2).reshape(
            128, KC * n).astype(bf))
    return tile(seg1, 480), tile(seg2, 384)


def kernel(hidden_states, wq, bq, wk, bk, wv, bv, wo, bo, frame, height, width):
    import ml_dtypes
    from concourse import bass_utils

    bf = ml_dtypes.bfloat16
    f, hh, ww = int(frame), int(height), int(width)
    hs = np.asarray(hidden_states, dtype=np.float32)
    assert hs.shape == (1, S, DIM) and f * hh * ww == S
    wq, wk, wv, wo = (np.asarray(a, np.float32) for a in (wq, wk, wv, wo))
    bq, bk, bv, bo = (np.asarray(a, np.float32) for a in (bq, bk, bv, bo))
    biased = bool(bq.any() or bk.any())

    perm = np.concatenate([np.arange(k, S, SPN) for k in range(SPN)])
    A, B = _rope_tables(f, hh, ww)
    A = _tile_rope(np.tile(A[perm], (1, 2)), bf)
    B = _tile_rope(np.tile(B[perm], (1, 2)), bf)
    hT = _tile_hT(hs[0].T[:, perm], bf)

    nc1 = _get(f"l1_{biased}", _build_launch1, biased)
    in1 = []
    for c in range(8):
        sl = slice(c * CW, (c + 1) * CW)
        w1, w2 = _pack_wall(wq, wk, wv, sl, bf)
        d = {
            "hT": hT,
            "wall1": w1,
            "wall2": w2,
            "bvi": np.concatenate(
                [np.concatenate([bv[sl][h * HD:(h + 1) * HD], [0.0]])
                 for h in range(HPC)]).astype(np.float32).reshape(1, -1),
            "A": A, "B": B,
        }
        if biased:
            d["bqk"] = np.concatenate(
                [np.concatenate([bq[sl][h * HD:(h + 1) * HD],
                                 bk[sl][h * HD:(h + 1) * HD]])
                 for h in range(HPC)]).astype(np.float32).reshape(1, -1)
        in1.append(d)
    td = os.environ.get("KERNEL_TRACE_DIR")
    if td:
        os.makedirs(td + "/l1", exist_ok=True)
        for fn in os.listdir(td + "/l1"):
            os.unlink(td + "/l1/" + fn)
    res1 = bass_utils.run_bass_kernel_spmd(
        nc1, in1, core_ids=list(range(8)),
        tmpdir=(td + "/l1") if td else None)
    LAST_RESULTS.append(res1)

    # outN [HPC, 8, 97, 512] bf16 -> [HPC*97, 4096] fp32, normalize, unpermute
    outN = np.concatenate(
        [np.asarray(res1.results[c]["outN"]).astype(np.float32)
         .transpose(0, 2, 1, 3).reshape(HPC, HD + 1, S) for c in range(8)], 0
    ).reshape(8 * HPC, HD + 1, S)
    attnT_g = (outN[:, :HD, :] / outN[:, HD:HD + 1, :]).reshape(DIM, S)
    attnT = np.empty_like(attnT_g)
    attnT[:, perm] = attnT_g

    biased2 = bool(bo.any())
    nc2 = _get(f"l2_{biased2}", _build_launch2, biased2)
    in2 = []
    for c in range(8):
        i, j = divmod(c, 2)
        d2 = {
            "attnT": np.ascontiguousarray(
                attnT[:, i * 1024:(i + 1) * 1024].astype(bf)),
            "woj": np.ascontiguousarray(
                wo[:, j * 1152:(j + 1) * 1152].astype(bf)),
        }
        if biased2:
            d2["boj"] = np.ascontiguousarray(
                bo[j * 1152:(j + 1) * 1152]).reshape(1, 1152)
        in2.append(d2)
    if td:
        os.makedirs(td + "/l2", exist_ok=True)
        for fn in os.listdir(td + "/l2"):
            os.unlink(td + "/l2/" + fn)
    res2 = bass_utils.run_bass_kernel_spmd(
        nc2, in2, core_ids=list(range(8)),
        tmpdir=(td + "/l2") if td else None)
    LAST_RESULTS.append(res2)

    out = np.empty((S, DIM), np.float32)
    for c in range(8):
        i, j = divmod(c, 2)
        out[i * 1024:(i + 1) * 1024, j * 1152:(j + 1) * 1152] = \
            np.asarray(res2.results[c]["out"]).T.astype(np.float32)
    return out[None]
